# revision 4
# baseline (speedup 1.0000x reference)
"""Cluster-memory cross-entropy loss on 8 Trainium2 NeuronCores.

loss = -mean_b log_softmax(normalize(inputs) @ features.T / T)[b, targets[b]]
  inputs [512,256] f32, features [65536,256] f32 (unit rows), targets [512].

Per core (8192-cluster shard of the 65536 columns):
  TensorE : fp8e4m3 DoubleRow matmuls — the full K=256 contraction in one
            MM (2 fp8 weights/cell).  lhsT = x-tile [128,2,128], rhs =
            feature slab [128,2,512], out [128 batch, 512 clusters] f32
            into one PSUM bank.  64 MMs/core; production order
            (jg-pair, m, jj) reloads weights only per (pair,m), and the
            LDWEIGHTS hide behind the MM stream in the PE reorder window.
  Regions : [128,1024] f32 PSUM tiles, pool bufs=4 (all 8 banks) so the
            MM->consumer->MM bank-recycle chain never binds.
  ScalarE : 19 regions via exp-spline activation with accum_out (exp and
            row-sum in ONE pass), matmul scale folded into the free affine.
  VectorE : 13 regions via Schraudolph fast-exp: int16(l*128*log2e*scale
            + 127*128 + .5 - 128*log2(c)) bitcast to bf16 IS ~2^t; summed
            by scalar_tensor_tensor on the two bf16 halves with accum_out.
            c = E[(1+f)2^-f] folds the mantissa bias into the affine, so
            the sums come out calibrated.
  DMA     : x-tiles on the ACT HWDGE ring; feature slabs on the SP ring
            in exact consumption order (FIFO), sized [512,512,1024,
            2048x3] so the first matmul starts as early as possible.
  Warmup  : 4 bf16 dummy matmuls on memset tiles run during the NEFF
            start barrier to flip the PE HAM clock gate to 2.4 GHz.
  Host    : combine per-region f32 sums, logZ = log(sumexp); target
            logits computed exactly in f64; loss = mean(logZ - t_logit).

fp8 scales S1 (x) and S2 (features) keep both operands ~unit-std in the
e4m3 normal range; quantization noise (~4% per logit) plus the bf16
Schraudolph error average out over 65536 clusters: measured end-to-end
relative error ~1.1e-4 against the fp32 reference (gate: 2e-2).
"""

import os
import numpy as np
import ml_dtypes

import concourse.bass as bass
import concourse.mybir as mybir
import concourse.tile as tile
from concourse import bacc
from concourse.bass_utils import run_bass_kernel_spmd

B, N, D, TEMP = 512, 65536, 256, 0.05
NCORES = 8
NSH = N // NCORES
MT = B // 128
RW = 1024                   # region width
NJB = NSH // RW             # 8 cluster blocks
NREG = NJB * MT             # 32 regions
NPAIR = NJB // 2            # 4 jg-pairs

S1, S2 = 4.0, 64.0
SCALE = 1.0 / (S1 * S2)

LOG2E = 1.4426950408889634
_f = np.linspace(0.0, 1.0, 1 << 17, endpoint=False) + 0.5 / (1 << 17)
SCH_C = float(np.mean((1.0 + _f) * np.exp2(-_f)))
SCH_A = 128.0 * LOG2E * SCALE
SCH_B = 127.0 * 128.0 + 0.5 - 128.0 * np.log2(SCH_C)

# all slabs on the sync HWDGE ring in consumption order: the ring is
# FIFO, so data lands in exactly the order the matmuls need it
SLAB_WIDTHS = [512, 512, 1024, 2048, 2048, 2048]
SLAB_STARTS = np.cumsum([0] + SLAB_WIDTHS).tolist()
SLAB_RING = ["sync"] * len(SLAB_WIDTHS)
assert SLAB_STARTS[-1] == NSH

XT_W = MT * 2 * 128
BLOB_W = XT_W + 2 * NSH

F32 = mybir.dt.float32
BF16 = mybir.dt.bfloat16
FP8 = mybir.dt.float8e4
I16 = mybir.dt.int16

# production order: q = pair*8 + m*2 + jj  (jg = pair*2 + jj)
def q_to_jgm(q):
    return (q // 8) * 2 + (q % 2), (q % 8) // 2


# 13 DVE regions spread evenly; region 31 stays on ACT for the tail
_d_idx = set(round(i * NREG / 13) for i in range(13))
PATTERN = os.environ.get(
    "K4_PATTERN", "".join("D" if q in _d_idx else "A" for q in range(NREG))
)
assert len(PATTERN) == NREG


def build_nc():
    nc = bacc.Bacc(target_bir_lowering=False, enable_partition_id=False)
    data = nc.declare_dram_parameter("data", [128, BLOB_W], FP8, isOutput=False)
    out = nc.declare_dram_parameter("out", [128, NREG], F32, isOutput=True)

    with tile.TileContext(nc) as tc:
        with (
            tc.tile_pool(name="sb", bufs=1) as sb_pool,
            tc.tile_pool(name="psum", bufs=4, space="PSUM") as psum_pool,
        ):
            tiny = sb_pool.tile([128, 1], F32)
            nc.vector.memset(tiny, 0.0)
            dumt = sb_pool.tile([128, 1], BF16)
            nc.scalar.activation(dumt[:], tiny[:], mybir.ActivationFunctionType.Exp)

            # xt rides FIRST in the sync-ring FIFO so its descriptors are
            # serviced before the slab packets crowd the SDMA queues
            xt_t = sb_pool.tile([128, MT, 2, 128], FP8)
            nc.scalar.dma_start(
                out=xt_t[:],
                in_=data[:, 0:XT_W].rearrange("p (m i c) -> p m i c", m=MT, i=2),
            )
            slabs = []
            for sl, w in enumerate(SLAB_WIDTHS):
                st = sb_pool.tile([128, 2, w], FP8, tag=f"slab{sl}", name=f"slab{sl}")
                off = XT_W + 2 * SLAB_STARTS[sl]
                eng = nc.sync if SLAB_RING[sl] == "sync" else nc.scalar
                eng.dma_start(
                    out=st[:],
                    in_=data[:, off : off + 2 * w].rearrange("p (i n) -> p i n", i=2),
                )
                slabs.append(st)

            def slab_of(col):
                for sl in range(len(SLAB_WIDTHS)):
                    if SLAB_STARTS[sl] <= col < SLAB_STARTS[sl + 1]:
                        return sl, col - SLAB_STARTS[sl]
                raise AssertionError(col)

            acc = sb_pool.tile([128, NREG], F32)

            # HAM warmup: bf16 dummies (128-col FWL weight loads are cheap
            # and hide), written into region q0's first bank, WAW-ordered
            # before the real start=True matmul
            wdum = sb_pool.tile([128, 128], BF16)
            rdum = sb_pool.tile([128, 512], BF16)
            nc.vector.memset(wdum, 0.0)
            nc.vector.memset(rdum, 0.0)
            ps0 = psum_pool.tile([128, RW], F32, tag="ps", name="ps_q0")
            for _ in range(4):
                nc.tensor.matmul(
                    ps0[:, 0:512], lhsT=wdum[:], rhs=rdum[:],
                    start=True, stop=True,
                )

            for q in range(NREG):
                jg, m = q_to_jgm(q)
                if q == 0:
                    ps = ps0
                else:
                    ps = psum_pool.tile([128, RW], F32, tag="ps", name=f"ps{q}")
                for g in range(RW // 512):
                    col = jg * RW + g * 512
                    sl, o = slab_of(col)
                    nc.tensor.matmul(
                        ps[:, g * 512 : (g + 1) * 512],
                        lhsT=xt_t[:, m],
                        rhs=slabs[sl][:, :, o : o + 512],
                        start=True,
                        stop=True,
                        perf_mode=mybir.MatmulPerfMode.DoubleRow,
                    )
                if PATTERN[q] == "A":
                    dume = sb_pool.tile([128, RW], BF16, bufs=2, tag="dume", name=f"de{q}")
                    nc.scalar.activation(
                        dume[:],
                        ps[:],
                        mybir.ActivationFunctionType.Exp,
                        scale=SCALE,
                        accum_out=acc[:, q : q + 1],
                    )
                else:
                    sint = sb_pool.tile([128, RW], I16, bufs=2, tag="sint", name=f"si{q}")
                    nc.vector.tensor_scalar(
                        sint[:], ps[:], SCH_A, SCH_B,
                        mybir.AluOpType.mult, mybir.AluOpType.add,
                    )
                    dumr = sb_pool.tile([128, RW // 2], BF16, bufs=2, tag="dumr", name=f"dr{q}")
                    nc.vector.scalar_tensor_tensor(
                        dumr[:],
                        sint[:, : RW // 2].bitcast(BF16),
                        1.0,
                        sint[:, RW // 2 :].bitcast(BF16),
                        mybir.AluOpType.mult,
                        mybir.AluOpType.add,
                        accum_out=acc[:, q : q + 1],
                    )
            nc.sync.dma_start(out=out[:, : NREG - 2], in_=acc[:, : NREG - 2])
            nc.sync.dma_start(out=out[:, NREG - 2 :], in_=acc[:, NREG - 2 :])
    nc.compile()
    return nc


_NC_CACHE = {}


def _get_nc():
    if "nc" not in _NC_CACHE:
        _NC_CACHE["nc"] = build_nc()
    return _NC_CACHE["nc"]


def prep_inputs(inputs, features):
    xn = inputs / np.linalg.norm(inputs, axis=1, keepdims=True)
    xs = (xn / TEMP).astype(np.float32)
    qx = (xs * S1).astype(ml_dtypes.float8_e4m3)
    xt_flat = np.ascontiguousarray(
        qx.reshape(MT, 128, 2, 128).transpose(3, 0, 2, 1)
    ).reshape(128, XT_W)
    qf = (features * S2).astype(ml_dtypes.float8_e4m3)
    blobs = []
    for cix in range(NCORES):
        fc = qf[cix * NSH : (cix + 1) * NSH]
        parts = [xt_flat]
        for sl, w in enumerate(SLAB_WIDTHS):
            s = SLAB_STARTS[sl]
            parts.append(
                np.ascontiguousarray(
                    fc[s : s + w].reshape(w, 2, 128).transpose(2, 1, 0)
                ).reshape(128, 2 * w)
            )
        blobs.append(np.ascontiguousarray(np.concatenate(parts, axis=1)))
    return xs, blobs


def run_cores(blobs, **kwargs):
    nc = _get_nc()
    in_maps = [{"data": blobs[c]} for c in range(NCORES)]
    return run_bass_kernel_spmd(nc, in_maps, list(range(NCORES)), **kwargs)


def combine(results, xs, features, targets):
    sumexp = np.zeros(B, dtype=np.float64)
    m_of_q = np.array([q_to_jgm(q)[1] for q in range(NREG)])
    for c in range(NCORES):
        o = results[c]["out"].astype(np.float64)  # [128, NREG]
        for m in range(MT):
            sumexp[m * 128 : (m + 1) * 128] += o[:, m_of_q == m].sum(axis=1)
    logz = np.log(sumexp)
    t_logit = (
        xs.astype(np.float64) * features[targets].astype(np.float64)
    ).sum(axis=1)
    return np.float32(np.mean(logz - t_logit))


def kernel(inputs, ema_inputs, targets, features):
    inputs = np.asarray(inputs, dtype=np.float32)
    features = np.asarray(features, dtype=np.float32)
    targets = np.asarray(targets)
    xs, blobs = prep_inputs(inputs, features)
    results = run_cores(blobs).results
    return combine(results, xs, features, targets)
